# revision 1
# baseline (speedup 1.0000x reference)
"""Trainium2 Bass kernel for DepthwiseSeparableConv3d (inference).

Problem: x[2,48,48,48,64] -> dw3x3x3 depthwise + BN + ReLU -> 1x1x1 conv
(64->128) + BN + ReLU -> z[2,48,48,48,128], all f32.

Strategy (8 NeuronCores, data-parallel over (b,d) slabs, 12 slabs/core):
 - Host pre-pads D (1-slab halo per side, zero at batch edges) and H/W
   (SAME padding) so the device kernel is a pure VALID 3x3x3 conv.
 - Depthwise conv runs on TensorE as a block-Toeplitz matmul:
   K=112 partitions = (8 channels x 14 W-inputs),
   M=96 partitions  = (8 channels x 12 W-outputs).
   The 3 W-taps live in the Toeplitz weight; the 9 (dz,dy) taps are
   PSUM-accumulated matmuls against free-dim-shifted views of the same
   SBUF tile (shifting free dims is free in an access pattern).
 - Weights are loaded once per tap-set via an explicit LDWEIGHTS; the
   matmuls are marked non-self-loading (ldweights=False) so the 8
   matmuls sharing a weight don't reload it.  Same for the pointwise
   weights (loaded exactly once).
 - BN1+ReLU is one ScalarE activation per (group, w-tile)
   (per-partition scale/bias).
 - A per-channel-group SBUF->SBUF DMA regroups (c,w)-partitions into
   pure-channel partitions (contiguous 576-elem blocks on both sides).
 - Pointwise 64->128 is a plain matmul; BN2+ReLU is one activation per
   4 position-chunks.
 - Output stays [f, positions] on device; host transposes to NDHWC.
"""

import os
import sys

for _p in ("/opt/trn_rl_repo", "/opt/pypackages"):
    if _p not in sys.path:
        sys.path.insert(0, _p)

import numpy as np
import ml_dtypes

import concourse.bass as bass
import concourse.tile as tile
from concourse import bacc, mybir
from concourse.bass_utils import run_bass_kernel_spmd

# ----- problem constants (hardcoded per spec) -----
B, D, H, W, C, F = 2, 48, 48, 48, 64, 128
EPS = 1e-3
N_CORES = 8
DPC = (B * D) // N_CORES          # d-slabs per core = 12
CG = 8                            # channels per depthwise group
NG = C // CG                      # 8 groups
WT = 4                            # W tiles
WO = W // WT                      # 12 outputs per W tile
WI = WO + 2                       # 14 inputs per W tile
KP = CG * WI                      # 112 K partitions
MP = CG * WO                      # 96 M partitions
DH = DPC * H                      # 576 (d,h) positions per W value
NHALF = 2                         # split (d,h) into two 288-col matmuls
NCOL = DH // NHALF                # 288
NPOS = DPC * H * W                # 27648 positions per core
ZCHUNK = 4                        # PW chunks per output DMA

BF16 = mybir.dt.bfloat16
F32 = mybir.dt.float32

_COMPILED = None


def _build_bass():
    nc = bacc.Bacc("TRN2", target_bir_lowering=False, debug=False,
                   num_devices=N_CORES)

    xt_d = nc.dram_tensor("xt", [NG, KP, WT, DPC + 2, H + 2], BF16,
                          kind="ExternalInput").ap()
    # wt packed k-major so all groups load in one DMA
    wt_d = nc.dram_tensor("wt", [KP, NG, 9, MP], BF16,
                          kind="ExternalInput").ap()
    pw_d = nc.dram_tensor("pwk", [C, F], BF16, kind="ExternalInput").ap()
    sb1_d = nc.dram_tensor("sb1", [MP, 2, NG], F32,
                           kind="ExternalInput").ap()
    sb2_d = nc.dram_tensor("sb2", [F, 2], F32, kind="ExternalInput").ap()
    z_d = nc.dram_tensor("z", [F, NPOS], F32, kind="ExternalOutput").ap()

    with tile.TileContext(nc) as tc:
        with (
            tc.tile_pool(name="consts", bufs=1) as consts,
            tc.tile_pool(name="xt", bufs=NG) as xt_pool,
            tc.tile_pool(name="wt", bufs=NG) as wt_pool,
            tc.tile_pool(name="ybuf", bufs=3) as y_pool,
            tc.tile_pool(name="Ybig", bufs=2) as Y_pool,
            tc.tile_pool(name="zbuf", bufs=2) as z_pool,
        ):
            pw_sb = consts.tile([C, F], BF16)
            sb1_sb = consts.tile([MP, 2, NG], F32)
            sb2_sb = consts.tile([F, 2], F32)

            # Y: depthwise output in pure-channel layout, one rotating
            # slice per w-tile (the PW of tile t runs during DW of t+1).

            xg = []
            for g in range(NG):
                xg_t = xt_pool.tile([KP, WT, DPC + 2, H + 2], BF16, tag="xg")
                xg.append(xg_t)
            wt_sb = wt_pool.tile([KP, NG, 9, MP], BF16)
            # load in need-order: the first matmul needs only g0's t0
            # slice and g0's weights — land those first, then the rest in
            # few big DMAs (HWDGE ring time is per-DMA)
            nc.sync.dma_start(xg[0][:, 0], xt_d[0, :, 0])
            nc.sync.dma_start(wt_sb[:, 0], wt_d[:, 0])
            nc.sync.dma_start(sb1_sb[:], sb1_d[:])
            nc.sync.dma_start(wt_sb[:, 1:NG], wt_d[:, 1:NG])
            nc.sync.dma_start(xg[1][:, 0], xt_d[1, :, 0])
            nc.sync.dma_start(xg[2][:, 0], xt_d[2, :, 0])
            nc.sync.dma_start(xg[3][:, 0], xt_d[3, :, 0])
            for g in range(4, NG):
                nc.sync.dma_start(xg[g][:, 0], xt_d[g, :, 0])
            for g in range(NG):
                nc.sync.dma_start(xg[g][:, 1:WT], xt_d[g, :, 1:WT])
                if g == 0:
                    nc.sync.dma_start(pw_sb[:], pw_d[:])
                    nc.sync.dma_start(sb2_sb[:], sb2_d[:])

            # per-tile flat position count and PW chunking
            TPOS = WO * DPC * H              # 6912
            NPW = 432                        # 16 uniform PW chunks per tile
            NQ = TPOS // NPW                 # 16
            zf = z_d.rearrange("f (t q n) -> f t q n", t=WT, q=NQ, n=NPW)

            with (
                tc.tile_pool(name="psdw", bufs=2, space="PSUM") as ps_pool,
                tc.tile_pool(name="pspw", bufs=2, space="PSUM") as pw_pool,
            ):
                Yt_tiles = {}
                pw_fifo = []

                def emit_pw_quad(t, q0):
                    Yt = Yt_tiles[t].rearrange("c w d h -> c (w d h)")
                    zt = z_pool.tile([F, 4, NPW], F32, tag="zt",
                                     name=f"zt_{t}_{q0}")
                    for half in range(2):
                        pps = pw_pool.tile([F, 2, 512], F32, tag="pwps",
                                           name=f"pps_{t}_{q0}_{half}")
                        for qq in range(2):
                            q = q0 + 2 * half + qq
                            nc.tensor.matmul(
                                pps[:, qq, 0:NPW], pw_sb[:],
                                Yt[:, q * NPW:(q + 1) * NPW],
                                start=True, stop=True)
                        nc.scalar.activation(
                            zt[:, 2 * half: 2 * half + 2, :],
                            pps[:, :, 0:NPW],
                            mybir.ActivationFunctionType.Relu,
                            bias=sb2_sb[:, 1:2], scale=sb2_sb[:, 0:1])
                    nc.sync.dma_start(zf[:, t, q0: q0 + 4, :], zt[:])

                for t in range(WT):
                    Yt_tiles[t] = Y_pool.tile([C, WO, DPC, H], BF16,
                                              tag="Yt", name=f"Yt_{t}")
                    for g in range(NG):
                        yg = y_pool.tile([MP, DPC, H], BF16, tag="yg")
                        ps = ps_pool.tile([MP, NHALF, 512], F32, tag="ps")
                        for izy, (dz, dy) in enumerate(
                                (a, b) for a in range(3) for b in range(3)):
                            for nh in range(NHALF):
                                d0 = nh * (DPC // NHALF)
                                rhs = xg[g][:, t,
                                            dz + d0: dz + d0 + DPC // NHALF,
                                            dy: dy + H]
                                nc.tensor.matmul(
                                    ps[:, nh, 0:NCOL],
                                    wt_sb[:, g, izy, :],
                                    rhs,
                                    start=(izy == 0),
                                    stop=(izy == 8),
                                )
                        out_v = yg[:].rearrange("c d h -> c (d h)") \
                                     .rearrange("c (n r) -> c n r",
                                                n=NHALF, r=NCOL)
                        nc.scalar.activation(
                            out_v,
                            ps[:, :, 0:NCOL],
                            mybir.ActivationFunctionType.Relu,
                            bias=sb1_sb[:, 1, g: g + 1],
                            scale=sb1_sb[:, 0, g: g + 1],
                        )
                        # regroup (c,w)-partitions -> channel partitions;
                        # on the ACT HWDGE ring (SP ring carries in/out)
                        nc.scalar.dma_start(
                            Yt_tiles[t][g * CG:(g + 1) * CG], yg[:])
                        # software pipeline: PW quads run one tile behind,
                        # staggered one slot deeper so the end-of-tile
                        # bridge latency always has ready PW work
                        if (g % 2 == 1 and pw_fifo
                                and not (t == WT - 1 and g == NG - 1)):
                            emit_pw_quad(*pw_fifo.pop(0))
                    for q0 in range(0, NQ, 4):
                        pw_fifo.append((t, q0))
                    if t == WT - 1:
                        while pw_fifo:
                            emit_pw_quad(*pw_fifo.pop(0))

    nc.compile()
    return nc


def _prep_inputs(x, dw_kernel, dw_bias, bn1_gamma, bn1_beta, bn1_mean,
                 bn1_var, pw_kernel, pw_bias, bn2_gamma, bn2_beta, bn2_mean,
                 bn2_var):
    """Build per-core input maps (numpy only, off the device clock)."""
    x = np.asarray(x, np.float32)
    dw_kernel = np.asarray(dw_kernel, np.float32)
    dw_bias = np.asarray(dw_bias, np.float32)
    bn1_gamma = np.asarray(bn1_gamma, np.float32)
    bn1_beta = np.asarray(bn1_beta, np.float32)
    bn1_mean = np.asarray(bn1_mean, np.float32)
    bn1_var = np.asarray(bn1_var, np.float32)
    pw_kernel = np.asarray(pw_kernel, np.float32)
    pw_bias = np.asarray(pw_bias, np.float32)
    bn2_gamma = np.asarray(bn2_gamma, np.float32)
    bn2_beta = np.asarray(bn2_beta, np.float32)
    bn2_mean = np.asarray(bn2_mean, np.float32)
    bn2_var = np.asarray(bn2_var, np.float32)
    a1 = bn1_gamma / np.sqrt(bn1_var + EPS)
    c1 = a1 * (dw_bias - bn1_mean) + bn1_beta
    a2 = bn2_gamma / np.sqrt(bn2_var + EPS)
    c2 = a2 * (pw_bias - bn2_mean) + bn2_beta

    # depthwise Toeplitz weights: [KP, NG, 9, MP] (k-major, one DMA)
    dw = dw_kernel[:, :, :, 0, :]                       # [3,3,3,C]
    wt = np.zeros((KP, NG, 9, MP), np.float32)
    for ci in range(CG):
        for wo in range(WO):
            for dx in range(3):
                # wt[ci*WI + wo+dx, g, (dz*3+dy), ci*WO + wo] = dw[dz,dy,dx,c]
                wt[ci * WI + wo + dx, :, :, ci * WO + wo] = (
                    dw[:, :, dx, :].reshape(9, NG, CG)[:, :, ci].T)
    wt = wt.astype(ml_dtypes.bfloat16)

    # BN1 scale/bias in (c-major, w) partition order: m = ci*WO + wo
    sb1 = np.zeros((MP, 2, NG), np.float32)
    for g in range(NG):
        for ci in range(CG):
            sb1[ci * WO:(ci + 1) * WO, 0, g] = a1[g * CG + ci]
            sb1[ci * WO:(ci + 1) * WO, 1, g] = c1[g * CG + ci]

    pwk = pw_kernel.astype(ml_dtypes.bfloat16)
    sb2 = np.stack([a2, c2], axis=1).astype(np.float32)   # [F, 2]

    # x padded once globally: [B, D+2, H+2, W+2, C]
    xp = np.zeros((B, D + 2, H + 2, W + 2, C), np.float32)
    xp[:, 1:-1, 1:-1, 1:-1, :] = x
    xp = xp.astype(ml_dtypes.bfloat16)

    in_maps = []
    for core in range(N_CORES):
        b = (core * DPC) // D
        d0 = (core * DPC) % D
        sl = xp[b, d0: d0 + DPC + 2]                    # [14, 50, 50, C]
        # xt[g, ci*WI+wi, t, d, h] = sl[d, h, 12t+wi, 8g+ci]
        xv = np.ascontiguousarray(sl.transpose(3, 2, 0, 1))  # [C, w50, d, h]
        # build overlapping w-tiles: index w = t*WO + wi
        idx = (np.arange(WT)[:, None] * WO + np.arange(WI)[None, :]).ravel()
        xv = xv[:, idx]                                 # [C, WT*WI, d, h]
        xt = xv.reshape(NG, CG, WT, WI, DPC + 2, H + 2) \
               .transpose(0, 1, 3, 2, 4, 5) \
               .reshape(NG, KP, WT, DPC + 2, H + 2)
        in_maps.append({
            "xt": np.ascontiguousarray(xt),
            "wt": wt, "pwk": pwk, "sb1": sb1, "sb2": sb2,
        })
    return in_maps


def _gather_output(results):
    z = np.empty((B, D, H, W, F), np.float32)
    for core in range(N_CORES):
        b = (core * DPC) // D
        d0 = (core * DPC) % D
        zc = results[core]["z"]                         # [F, NPOS]
        # free order was (t, w_o, d, h); w_global = t*WO + w_o
        zc = zc.reshape(F, WT, WO, DPC, H).transpose(3, 4, 1, 2, 0)
        z[b, d0: d0 + DPC] = zc.reshape(DPC, H, W, F)
    return z


def kernel(**inputs):
    global _COMPILED
    if _COMPILED is None:
        _COMPILED = _build_bass()
    in_maps = _prep_inputs(**inputs)
    res = run_bass_kernel_spmd(_COMPILED, in_maps,
                               core_ids=list(range(N_CORES)))
    return _gather_output(res.results)


if __name__ == "__main__":
    pass

